# revision 3
# baseline (speedup 1.0000x reference)
"""v4: v2 + the d intermediate lives in PSUM (padded to 5 banks),
moving 16B/elem of d traffic off the SBUF ports.

out_pad[i] = y[i] + a[i]*d[i-1] + b[i]*d[i] + c[i]*d[i+1],  d = diff(y)
with a = -w0, b = w2+w3, c = w3 (host-scattered).  Device emits
s1 = a*d(-1) + b*d(0)  and  s2 = c*d(+1); host adds y0v + s1 + s2 at the
gathered positions.
"""
import os

os.environ.setdefault("NEURON_RT_VIRTUAL_CORE_SIZE", "1")

import numpy as np

N_CH = 32
N_IN = 1_048_576
N_OUT = 963_380
N_CORES = 8
B = N_IN // N_CORES          # 131072 input samples per core
P = 128
F = B // P                   # 1024 free-dim cols per partition row
NG = N_CH // 2               # 16 two-channel groups
HALO = 3
WL = B + HALO                # 131075 window elems per channel
WLP = 131088                 # padded to 16-elem (64B) multiple

UP_BUFS = 4
D_BUFS = 3
M_BUFS = 3
S_BUFS = 3


def _hermite_weights(x64: np.ndarray) -> np.ndarray:
    """4 Niemitalo weights per output sample, [4, n] float64."""
    x2 = x64 * x64
    x3 = x2 * x64
    return np.stack(
        [
            -0.5 * x3 + x2 - 0.5 * x64,
            1.5 * x3 - 2.5 * x2 + 1.0,
            -1.5 * x3 + 2.0 * x2 + 0.5 * x64,
            0.5 * x3 - 0.5 * x2,
        ],
        0,
    )


def _build_device_kernel(reps=1):
    import concourse.bacc as bacc
    import concourse.mybir as mybir
    import concourse.tile as tile
    from concourse.ap import AP

    nc = bacc.Bacc(
        "TRN2",
        target_bir_lowering=False,
        debug=False,
        enable_asserts=False,
        num_devices=N_CORES,
    )
    dt = mybir.dt.float32
    y_d = nc.dram_tensor("yw", [N_CH, WLP], dt, kind="ExternalInput").ap()
    w_d = nc.dram_tensor("w", [3, P, F], dt, kind="ExternalInput").ap()
    o_d = nc.dram_tensor("o", [NG, P, 2, F], dt, kind="ExternalOutput").ap()
    o2_d = nc.dram_tensor("o2", [NG, P, 2, F], dt, kind="ExternalOutput").ap()

    mult = mybir.AluOpType.mult
    add = mybir.AluOpType.add
    sub = mybir.AluOpType.subtract

    with tile.TileContext(nc) as tc:
        with (
            tc.tile_pool(name="wp", bufs=1) as wp,
            tc.tile_pool(name="up", bufs=UP_BUFS) as up,
            tc.psum_pool(name="dp", bufs=1) as dp,
            tc.tile_pool(name="mp", bufs=M_BUFS) as mp,
            tc.tile_pool(name="sp", bufs=S_BUFS) as sp,
        ):
            wt = []
            w_tiles = []
            for t in range(3):
                w_tile = wp.tile([P, F], dt, tag=f"w{t}")
                w_tiles.append(w_tile)
                wt.append(w_tile[:].unsqueeze(1).broadcast_to([P, 2, F]))
            nc.scalar.dma_start(out=w_tiles[0][:], in_=w_d[0])
            nc.scalar.dma_start(out=w_tiles[1][:], in_=w_d[1])
            for g in [g for _ in range(reps) for g in range(NG)]:
                yt = up.tile([P, 2, F + HALO], dt, tag="y")
                src = AP(y_d.tensor, 2 * g * WLP,
                         [(F, P), (WLP, 2), (1, F + HALO)])
                ldeng = nc.sync if g % 2 == 0 else nc.scalar
                ldeng.dma_start(out=yt[:], in_=src)
                if g == 0:
                    nc.sync.dma_start(out=w_tiles[2][:], in_=w_d[2])
                dtl = dp.tile([P, 2, F + 2], dt, tag="d",
                              padded_shape=[P, 2, 1280])
                nc.vector.tensor_tensor(
                    out=dtl[:], in0=yt[:, :, 1:F + 3], in1=yt[:, :, 0:F + 2],
                    op=sub)
                m1 = mp.tile([P, 2, F], dt, tag="m1")
                nc.vector.tensor_tensor(
                    out=m1[:], in0=dtl[:, :, 0:F], in1=wt[0], op=mult)
                m2 = mp.tile([P, 2, F], dt, tag="m2")
                nc.vector.tensor_tensor(
                    out=m2[:], in0=dtl[:, :, 1:F + 1], in1=wt[1], op=mult)
                s1 = sp.tile([P, 2, F], dt, tag="s1")
                nc.vector.tensor_tensor(
                    out=s1[:], in0=m1[:], in1=m2[:], op=add)
                nc.sync.dma_start(out=o_d[g], in_=s1[:])
                s2 = sp.tile([P, 2, F], dt, tag="s2")
                nc.vector.tensor_tensor(
                    out=s2[:], in0=dtl[:, :, 2:F + 2], in1=wt[2], op=mult)
                nc.scalar.dma_start(out=o2_d[g], in_=s2[:])
    nc.compile()
    return nc


_NC_CACHE = None


def _get_nc():
    global _NC_CACHE
    if _NC_CACHE is None:
        _NC_CACHE = _build_device_kernel()
    return _NC_CACHE


def _check_structure(y0, ym1, y1, y2):
    d = np.diff(y0)
    if d.size == 0 or not (d.min() >= 1 and d.max() <= 2):
        return False
    if not np.array_equal(ym1, np.maximum(y0 - 1, 0)):
        return False
    if not np.array_equal(y1, np.minimum(y0 + 1, N_IN - 1)):
        return False
    return np.array_equal(y2, np.minimum(y1 + 1, N_IN - 1))


def _prep_inputs(y, x, y_m1_idx, y0_idx, y1_idx, y2_idx):
    """Host-side restructure. Returns (in_maps, y0), or None when the
    indices don't match the resampler pattern (caller falls back)."""
    y = np.ascontiguousarray(np.asarray(y, dtype=np.float32))
    y0 = np.asarray(y0_idx, dtype=np.int64)
    if y.shape != (N_CH, N_IN) or y0.shape != (N_OUT,):
        return None
    if not _check_structure(
        y0,
        np.asarray(y_m1_idx, dtype=np.int64),
        np.asarray(y1_idx, dtype=np.int64),
        np.asarray(y2_idx, dtype=np.int64),
    ):
        return None
    wk = _hermite_weights(np.asarray(x, dtype=np.float64))  # [4, N_OUT] f64
    abc = np.stack([-wk[0], wk[2] + wk[3], wk[3]], 0).astype(np.float32)
    # scatter weights onto the input grid (y0 strictly increasing)
    W = np.zeros((3, N_IN), np.float32)
    W[:, y0] = abc
    # edge-replicated input for halo taps
    ypad = np.pad(y, ((0, 0), (1, 2)), mode="edge")  # [32, N_IN+3]
    in_maps = []
    for ci in range(N_CORES):
        i0 = B * ci
        yw = np.zeros((N_CH, WLP), np.float32)
        yw[:, :WL] = ypad[:, i0:i0 + WL]
        Wl = np.ascontiguousarray(W[:, i0:i0 + B].reshape(3, P, F))
        in_maps.append({"yw": yw, "w": Wl})
    return in_maps, y0


def _assemble(results, y0, y):
    op = np.empty((N_CH, N_IN), np.float32)
    for ci, res in enumerate(results):
        o = res["o"] + res["o2"]
        o = o.reshape(NG, P, 2, F).transpose(0, 2, 1, 3)
        op[:, B * ci:B * (ci + 1)] = o.reshape(N_CH, B)
    return np.ascontiguousarray(op[:, y0] + y[:, y0])


def run_on_device(in_maps, trace=False):
    from concourse import bass_utils

    nc = _get_nc()
    return bass_utils.run_bass_kernel_spmd(
        nc, in_maps, core_ids=list(range(N_CORES)), trace=trace
    )


def _fallback(y, x, y_m1_idx, y0_idx, y1_idx, y2_idx):
    """Generic-index path (never hit for the real resampler inputs)."""
    y = np.asarray(y, np.float32)
    x = np.asarray(x, np.float32)
    ym1 = y[:, np.asarray(y_m1_idx, np.int64)]
    y0v = y[:, np.asarray(y0_idx, np.int64)]
    y1v = y[:, np.asarray(y1_idx, np.int64)]
    y2v = y[:, np.asarray(y2_idx, np.int64)]
    c1 = np.float32(0.5) * (y1v - ym1)
    c2 = ym1 - np.float32(2.5) * y0v + np.float32(2.0) * y1v \
        - np.float32(0.5) * y2v
    c3 = np.float32(0.5) * (y2v - ym1) + np.float32(1.5) * (y0v - y1v)
    return ((c3 * x + c2) * x + c1) * x + y0v


def kernel(y, x, y_m1_idx, y0_idx, y1_idx, y2_idx):
    prep = _prep_inputs(y, x, y_m1_idx, y0_idx, y1_idx, y2_idx)
    if prep is None:
        return _fallback(y, x, y_m1_idx, y0_idx, y1_idx, y2_idx)
    in_maps, y0 = prep
    r = run_on_device(in_maps, trace=False)
    return _assemble(r.results, y0, np.asarray(y, np.float32))


# revision 4
# speedup vs baseline: 6.2940x; 6.2940x over previous
"""v2: diff-form stencil — 5 DVE passes (vs 6) for the same 2-stream output.

out_pad[i] = y[i] + a[i]*d[i-1] + b[i]*d[i] + c[i]*d[i+1],  d = diff(y)
with a = -w0, b = w2+w3, c = w3 (host-scattered).  Device emits
s1 = a*d(-1) + b*d(0)  and  s2 = c*d(+1); host adds y0v + s1 + s2 at the
gathered positions.
"""
import os

os.environ.setdefault("NEURON_RT_VIRTUAL_CORE_SIZE", "1")

import numpy as np

N_CH = 32
N_IN = 1_048_576
N_OUT = 963_380
N_CORES = 8
B = N_IN // N_CORES          # 131072 input samples per core
P = 128
F = B // P                   # 1024 free-dim cols per partition row
NG = N_CH // 2               # 16 two-channel groups
HALO = 3
WL = B + HALO                # 131075 window elems per channel
WLP = 131088                 # padded to 16-elem (64B) multiple

UP_BUFS = 4
D_BUFS = 3
M_BUFS = 3
S_BUFS = 3


def _hermite_weights(x64: np.ndarray) -> np.ndarray:
    """4 Niemitalo weights per output sample, [4, n] float64."""
    x2 = x64 * x64
    x3 = x2 * x64
    return np.stack(
        [
            -0.5 * x3 + x2 - 0.5 * x64,
            1.5 * x3 - 2.5 * x2 + 1.0,
            -1.5 * x3 + 2.0 * x2 + 0.5 * x64,
            0.5 * x3 - 0.5 * x2,
        ],
        0,
    )


def _build_device_kernel(reps=1):
    import concourse.bacc as bacc
    import concourse.mybir as mybir
    import concourse.tile as tile
    from concourse.ap import AP

    nc = bacc.Bacc(
        "TRN2",
        target_bir_lowering=False,
        debug=False,
        enable_asserts=False,
        num_devices=N_CORES,
    )
    dt = mybir.dt.float32
    y_d = nc.dram_tensor("yw", [N_CH, WLP], dt, kind="ExternalInput").ap()
    w_d = nc.dram_tensor("w", [3, P, F], dt, kind="ExternalInput").ap()
    o_d = nc.dram_tensor("o", [NG, P, 2, F], dt, kind="ExternalOutput").ap()
    o2_d = nc.dram_tensor("o2", [NG, P, 2, F], dt, kind="ExternalOutput").ap()

    mult = mybir.AluOpType.mult
    add = mybir.AluOpType.add
    sub = mybir.AluOpType.subtract

    with tile.TileContext(nc) as tc:
        with (
            tc.tile_pool(name="wp", bufs=1) as wp,
            tc.tile_pool(name="up", bufs=UP_BUFS) as up,
            tc.tile_pool(name="dp", bufs=D_BUFS) as dp,
            tc.tile_pool(name="mp", bufs=M_BUFS) as mp,
            tc.tile_pool(name="sp", bufs=S_BUFS) as sp,
        ):
            wt = []
            w_tiles = []
            for t in range(3):
                w_tile = wp.tile([P, F], dt, tag=f"w{t}")
                w_tiles.append(w_tile)
                wt.append(w_tile[:].unsqueeze(1).broadcast_to([P, 2, F]))
            nc.scalar.dma_start(out=w_tiles[0][:], in_=w_d[0])
            nc.scalar.dma_start(out=w_tiles[1][:], in_=w_d[1])
            for g in [g for _ in range(reps) for g in range(NG)]:
                yt = up.tile([P, 2, F + HALO], dt, tag="y")
                src = AP(y_d.tensor, 2 * g * WLP,
                         [(F, P), (WLP, 2), (1, F + HALO)])
                ldeng = nc.sync if g % 2 == 0 else nc.scalar
                ldeng.dma_start(out=yt[:], in_=src)
                if g == 0:
                    nc.sync.dma_start(out=w_tiles[2][:], in_=w_d[2])
                dtl = dp.tile([P, 2, F + 2], dt, tag="d")
                nc.vector.tensor_tensor(
                    out=dtl[:], in0=yt[:, :, 1:F + 3], in1=yt[:, :, 0:F + 2],
                    op=sub)
                m1 = mp.tile([P, 2, F], dt, tag="m1")
                nc.vector.tensor_tensor(
                    out=m1[:], in0=dtl[:, :, 0:F], in1=wt[0], op=mult)
                m2 = mp.tile([P, 2, F], dt, tag="m2")
                nc.vector.tensor_tensor(
                    out=m2[:], in0=dtl[:, :, 1:F + 1], in1=wt[1], op=mult)
                s1 = sp.tile([P, 2, F], dt, tag="s1")
                nc.vector.tensor_tensor(
                    out=s1[:], in0=m1[:], in1=m2[:], op=add)
                nc.sync.dma_start(out=o_d[g], in_=s1[:])
                s2 = sp.tile([P, 2, F], dt, tag="s2")
                nc.vector.tensor_tensor(
                    out=s2[:], in0=dtl[:, :, 2:F + 2], in1=wt[2], op=mult)
                nc.scalar.dma_start(out=o2_d[g], in_=s2[:])
    nc.compile()
    return nc


_NC_CACHE = None


def _get_nc():
    global _NC_CACHE
    if _NC_CACHE is None:
        _NC_CACHE = _build_device_kernel()
    return _NC_CACHE


def _check_structure(y0, ym1, y1, y2):
    d = np.diff(y0)
    if d.size == 0 or not (d.min() >= 1 and d.max() <= 2):
        return False
    if not np.array_equal(ym1, np.maximum(y0 - 1, 0)):
        return False
    if not np.array_equal(y1, np.minimum(y0 + 1, N_IN - 1)):
        return False
    return np.array_equal(y2, np.minimum(y1 + 1, N_IN - 1))


def _prep_inputs(y, x, y_m1_idx, y0_idx, y1_idx, y2_idx):
    """Host-side restructure. Returns (in_maps, y0), or None when the
    indices don't match the resampler pattern (caller falls back)."""
    y = np.ascontiguousarray(np.asarray(y, dtype=np.float32))
    y0 = np.asarray(y0_idx, dtype=np.int64)
    if y.shape != (N_CH, N_IN) or y0.shape != (N_OUT,):
        return None
    if not _check_structure(
        y0,
        np.asarray(y_m1_idx, dtype=np.int64),
        np.asarray(y1_idx, dtype=np.int64),
        np.asarray(y2_idx, dtype=np.int64),
    ):
        return None
    wk = _hermite_weights(np.asarray(x, dtype=np.float64))  # [4, N_OUT] f64
    abc = np.stack([-wk[0], wk[2] + wk[3], wk[3]], 0).astype(np.float32)
    # scatter weights onto the input grid (y0 strictly increasing)
    W = np.zeros((3, N_IN), np.float32)
    W[:, y0] = abc
    # edge-replicated input for halo taps
    ypad = np.pad(y, ((0, 0), (1, 2)), mode="edge")  # [32, N_IN+3]
    in_maps = []
    for ci in range(N_CORES):
        i0 = B * ci
        yw = np.zeros((N_CH, WLP), np.float32)
        yw[:, :WL] = ypad[:, i0:i0 + WL]
        Wl = np.ascontiguousarray(W[:, i0:i0 + B].reshape(3, P, F))
        in_maps.append({"yw": yw, "w": Wl})
    return in_maps, y0


def _assemble(results, y0, y):
    op = np.empty((N_CH, N_IN), np.float32)
    for ci, res in enumerate(results):
        o = res["o"] + res["o2"]
        o = o.reshape(NG, P, 2, F).transpose(0, 2, 1, 3)
        op[:, B * ci:B * (ci + 1)] = o.reshape(N_CH, B)
    return np.ascontiguousarray(op[:, y0] + y[:, y0])


def run_on_device(in_maps, trace=False):
    from concourse import bass_utils

    nc = _get_nc()
    return bass_utils.run_bass_kernel_spmd(
        nc, in_maps, core_ids=list(range(N_CORES)), trace=trace
    )


def _fallback(y, x, y_m1_idx, y0_idx, y1_idx, y2_idx):
    """Generic-index path (never hit for the real resampler inputs)."""
    y = np.asarray(y, np.float32)
    x = np.asarray(x, np.float32)
    ym1 = y[:, np.asarray(y_m1_idx, np.int64)]
    y0v = y[:, np.asarray(y0_idx, np.int64)]
    y1v = y[:, np.asarray(y1_idx, np.int64)]
    y2v = y[:, np.asarray(y2_idx, np.int64)]
    c1 = np.float32(0.5) * (y1v - ym1)
    c2 = ym1 - np.float32(2.5) * y0v + np.float32(2.0) * y1v \
        - np.float32(0.5) * y2v
    c3 = np.float32(0.5) * (y2v - ym1) + np.float32(1.5) * (y0v - y1v)
    return ((c3 * x + c2) * x + c1) * x + y0v


def kernel(y, x, y_m1_idx, y0_idx, y1_idx, y2_idx):
    prep = _prep_inputs(y, x, y_m1_idx, y0_idx, y1_idx, y2_idx)
    if prep is None:
        return _fallback(y, x, y_m1_idx, y0_idx, y1_idx, y2_idx)
    in_maps, y0 = prep
    r = run_on_device(in_maps, trace=False)
    return _assemble(r.results, y0, np.asarray(y, np.float32))
